# revision 2
# baseline (speedup 1.0000x reference)
"""Causal MHA (b=2,h=32,s=2048,d=128) on 8 TRN2 cores — v3.

Device computes unnormalized attention: ctx_u^T = exp(S^T/sqrt(d)) @ V and
partial softmax denominators; the host does the final (cheap, O(b s h d))
normalization ctx_u / l. This removes the per-q-block cleanup/offset
ones-matmuls from the PE, the reciprocal and final multiply from the DVE,
and frees the l PSUM bank so S chunks are uniform [128,1536] double-buffered
(96 ACT insts/core).

Device outputs per head:
  out   [128, 2048] bf16 — unnormalized ctx^T
  l2o   [4, 128, 512] bf16 — DVE fold-tree partial sums of the full-width
        P slots per q-block (host: l_j = sum over partitions + offset part)
  pofs  [6, 128, 512] bf16 — the 6 slots holding offset (384/128/256) pieces
        (host adds their column sums into l at the pieces' q-offsets)

Per-head stream: 34 slots x 512 packed S^T cols (see _plan_head).
"""
import math
import sys

if '/opt/trn_rl_repo' not in sys.path:
    sys.path.insert(0, '/opt/trn_rl_repo')

import numpy as np
import ml_dtypes

import concourse.bass as bass
import concourse.tile as tile
from concourse import mybir, bacc
from concourse.bass_utils import run_bass_kernel_spmd

F32 = mybir.dt.float32
BF16 = mybir.dt.bfloat16
EXP = mybir.ActivationFunctionType.Exp
MULT = mybir.AluOpType.mult
ADD = mybir.AluOpType.add

B, H, S, D = 2, 32, 2048, 128
N_CORES = 8
HPC = (B * H) // N_CORES
QB = 512
NQB = S // QB
NKT = S // 128
SCALE = 1.0 / math.sqrt(D)
NSLOT = 34
RING = NSLOT * 512
NOFS = 6                     # offset-piece slots per head, DMA'd for host


def _plan_head():
    slots, cur = [], []
    cur_w = 0
    tri_slots = {}
    runs = {}
    ofs_slots = []           # slot indices DMA'd to pofs

    def put(j, t, qo, w, defer=False):
        nonlocal cur, cur_w
        cur.append(dict(j=j, t=t, qo=qo, w=w, so=cur_w, defer=defer,
                        offset=(w < 512)))
        if t >= 4 * j:
            tri_slots.setdefault(len(slots), []).append(cur_w)
        if w < 512 and (not ofs_slots or ofs_slots[-1] != len(slots)):
            ofs_slots.append(len(slots))
        cur_w += w
        assert cur_w <= 512
        if cur_w == 512:
            slots.append(cur)
            cur, cur_w = [], 0

    for j in range(NQB):
        base = 4 * j
        run_start = len(slots)
        for t in range(base):
            put(j, t, 0, 512)
        put(j, base, 0, 512)
        runs[j] = (run_start, len(slots) - run_start)
        put(j, base + 1, 128, 384)
        put(j, base + 3, 384, 128)
        if j in (0, 2):
            put(j, base + 2, 256, 256)
            jn = j + 1
            put(jn, 4 * jn + 2, 256, 256, defer=True)
    assert cur_w == 0 and len(slots) == NSLOT and len(ofs_slots) == NOFS

    order = [p for sl in slots for p in sl]
    last = {}
    for i, p in enumerate(order):
        last[p["j"]] = i
    for i, p in enumerate(order):
        p["stop"] = (last[p["j"]] == i)
    return slots, runs, tri_slots, ofs_slots


def _build(n_heads=HPC, la_chunks=2):
    nc = bacc.Bacc("TRN2", target_bir_lowering=False, debug=False,
                   num_devices=N_CORES)
    qt = nc.dram_tensor("qt", [n_heads, 128, S], BF16, kind="ExternalInput")
    kt = nc.dram_tensor("kt", [n_heads, 128, S], BF16, kind="ExternalInput")
    v = nc.dram_tensor("v", [n_heads, S, D], BF16, kind="ExternalInput")
    tri = nc.dram_tensor("tri", [128, 128], BF16, kind="ExternalInput")
    out = nc.dram_tensor("out", [n_heads, 128, S], BF16, kind="ExternalOutput")
    l2o = nc.dram_tensor("l2o", [n_heads, NQB, 128, QB], BF16,
                         kind="ExternalOutput")
    pofs = nc.dram_tensor("pofs", [n_heads, NOFS, 128, QB], BF16,
                          kind="ExternalOutput")

    slots_h, runs_h, tri_h, ofs_slots = _plan_head()
    ofs_idx = {s: i for i, s in enumerate(ofs_slots)}
    # per-head chunks: 10x3 + 2x2 (last two padded into the 1536 tags)
    head_chunks = []
    s0 = 0
    for n in [3] * 10 + [2, 2]:
        head_chunks.append((s0, n))
        s0 += n
    assert s0 == NSLOT

    # incremental l2 accumulation: per head-chunk, the run slots it contains
    # (j, first_slot_in_chunk_rel, count, first_of_run, last_of_run)
    run_parts = {ci: [] for ci in range(len(head_chunks))}
    for j, (rs, rn) in runs_h.items():
        for ci, (cs, n) in enumerate(head_chunks):
            a = max(rs, cs)
            b = min(rs + rn, cs + n)
            if a < b:
                run_parts[ci].append(
                    (j, a - cs, b - a, a == rs, b == rs + rn))

    chunks = []
    for h in range(n_heads):
        for (cs, n) in head_chunks:
            chunks.append((h, cs, n))

    with tile.TileContext(nc) as tc:
        with (tc.tile_pool(name="heads", bufs=2) as hp,
              tc.tile_pool(name="consts", bufs=1) as cp,
              tc.tile_pool(name="pring", bufs=1) as prp,
              tc.tile_pool(name="tmps", bufs=2) as tp,
              tc.tile_pool(name="outp", bufs=2) as outp,
              tc.tile_pool(name="ps_s", bufs=1, space="PSUM") as ps_s,
              tc.tile_pool(name="ps_c", bufs=2, space="PSUM") as ps_c):
            tri_sb = cp.tile([128, 128], BF16)
            nc.sync.dma_start(tri_sb, tri[:, :])
            p_ring = prp.tile([128, RING], BF16, name="p_ring")

            head_sb = {}

            def prep_head(h):
                if h in head_sb:
                    return head_sb[h]
                qt_sb = hp.tile([128, S], BF16, tag="qt", name="qt_sb")
                kt_sb = hp.tile([128, S], BF16, tag="kt", name="kt_sb")
                v_sb = hp.tile([128, NKT, D], BF16, tag="v", name="v_sb")
                # first 512 cols land first so the head's first matmuls start
                # early; the rest in one big transfer each
                nc.sync.dma_start(kt_sb[:, :512], kt[h, :, :512])
                nc.sync.dma_start(qt_sb[:, :512], qt[h, :, :512])
                nc.sync.dma_start(kt_sb[:, 512:], kt[h, :, 512:])
                nc.sync.dma_start(qt_sb[:, 512:], qt[h, :, 512:])
                vr = v[h].rearrange("(t p) d -> p t d", p=128)
                for t0 in range(0, NKT, 8):
                    nc.sync.dma_start(v_sb[:, t0:t0 + 8, :], vr[:, t0:t0 + 8, :])
                head_sb[h] = (qt_sb, kt_sb, v_sb)
                return head_sb[h]

            def emit_s_chunk(ci):
                h, cs, n = chunks[ci]
                qt_sb, kt_sb, _ = prep_head(h)
                ci_h = ci % len(head_chunks)
                s_ps = ps_s.tile([128, 1536], F32, tag=f"s{ci_h % 2}",
                                 name="s_ps")
                for si in range(n):
                    for p in slots_h[cs + si]:
                        col = si * 512 + p["so"]
                        j, t = p["j"], p["t"]
                        nc.tensor.matmul(
                            s_ps[:, col:col + p["w"]],
                            kt_sb[:, t * 128:(t + 1) * 128],
                            qt_sb[:, j * QB + p["qo"]:j * QB + p["qo"] + p["w"]],
                            start=True, stop=True)
                return s_ps

            ctx_ps = {}
            deferred = {}
            npieces = {}
            for h in range(n_heads):
                for sl in slots_h:
                    for p in sl:
                        npieces[(h, p["j"])] = npieces.get((h, p["j"]), 0) + 1

            def emit_ctx(h, p, rcol, start):
                j, t = p["j"], p["t"]
                _, _, v_sb = head_sb[h]
                nc.tensor.matmul(
                    ctx_ps[(h, j)][:, p["qo"]:p["qo"] + p["w"]],
                    v_sb[:, t, :], p_ring[:, rcol:rcol + p["w"]],
                    start=start, stop=p["stop"])

            l2acc = {}    # (h, j) -> acc tile [128,512] bf16

            def head_stage(h):
                if h not in stage:
                    stage[h] = (
                        outp.tile([128, S], BF16, tag="ctx_out",
                                  name="ctx_stage"),
                        outp.tile([128, S], BF16, tag="l2s",
                                  name="l2_stage"))
                return stage[h]

            def emit_l2(h, ci_h, rc0):
                """Incremental l2 accumulation for this chunk's run slots.
                The run's final op writes the head's l2 staging tile."""
                for (j, rel, cnt, first, last) in run_parts[ci_h]:
                    key = (h, j)
                    c0 = rc0 + rel * 512
                    dest = head_stage(h)[1][:, j * QB:(j + 1) * QB]
                    srcs = [p_ring[:, c0 + k * 512:c0 + (k + 1) * 512]
                            for k in range(cnt)]
                    if first:
                        acc = l2acc[key] = tp.tile([128, QB], BF16, tag="acc",
                                                   name="l2_acc", bufs=3)
                        if cnt == 1:
                            if last:
                                nc.vector.tensor_copy(dest, srcs[0])
                            else:
                                nc.vector.tensor_copy(acc, srcs[0])
                            srcs = []
                        else:
                            nc.vector.tensor_tensor(out=acc, in0=srcs[0],
                                                    in1=srcs[1], op=ADD)
                            srcs = srcs[2:]
                    else:
                        acc = l2acc[key]
                        if cnt == 2:
                            tf = tp.tile([128, QB], BF16, tag="tf", name="tf")
                            nc.vector.tensor_tensor(out=tf, in0=srcs[0],
                                                    in1=srcs[1], op=ADD)
                            srcs = [tf]
                    for si, sap in enumerate(srcs):
                        o = dest if (last and si == len(srcs) - 1) else acc
                        nc.vector.tensor_tensor(out=o, in0=acc, in1=sap,
                                                op=ADD)
                    if last:
                        l2acc.pop(key, None)

            def process_piece(h, p, rcol):
                key = (h, p["j"])
                if p["defer"]:
                    deferred.setdefault(key, []).append((p, rcol))
                    finish_piece(h, p)
                    return
                if p["t"] == 0:
                    assert key not in ctx_ps
                    ctx_ps[key] = ps_c.tile([128, QB], F32, tag="ctx",
                                            name="ctx_ps")
                    emit_ctx(h, p, rcol, start=True)
                    for (dp, drcol) in deferred.pop(key, []):
                        emit_ctx(h, dp, drcol, start=False)
                else:
                    assert key in ctx_ps
                    emit_ctx(h, p, rcol, start=False)
                finish_piece(h, p)

            stage = {}

            def finish_piece(h, p):
                key = (h, p["j"])
                npieces[key] -= 1
                if npieces[key] == 0:
                    j = p["j"]
                    ctx_st, l2_st = head_stage(h)
                    nc.vector.tensor_copy(ctx_st[:, j * QB:(j + 1) * QB],
                                          ctx_ps[key][:, :])
                    del ctx_ps[key]
                    if j == NQB - 1:
                        nc.sync.dma_start(out[h], ctx_st)
                        nc.sync.dma_start(
                            l2o[h].rearrange("t p c -> p t c"),
                            l2_st.rearrange("p (t c) -> p t c", t=NQB))
                        del stage[h]

            pending = [emit_s_chunk(ci) for ci in range(min(la_chunks, len(chunks)))]

            for ci in range(len(chunks)):
                if ci + la_chunks < len(chunks):
                    pending.append(emit_s_chunk(ci + la_chunks))
                s_ps = pending.pop(0)
                h, cs, n = chunks[ci]
                rc0 = cs * 512
                nc.scalar.activation(p_ring[:, rc0:rc0 + n * 512],
                                     s_ps[:, :n * 512], EXP, scale=SCALE)
                for si in range(n):
                    tcols = tri_h.get(cs + si, [])
                    scol = rc0 + si * 512
                    if len(tcols) == 2:
                        stride = tcols[1] - tcols[0]
                        pap = bass.AP(tensor=p_ring.tensor,
                                      offset=p_ring.offset + scol + tcols[0],
                                      ap=[p_ring.ap[0], [stride, 2], [1, 128]])
                        tap = bass.AP(tensor=tri_sb.tensor,
                                      offset=tri_sb.offset,
                                      ap=[tri_sb.ap[0], [0, 2], [1, 128]])
                        nc.gpsimd.tensor_tensor(out=pap, in0=pap, in1=tap,
                                                op=MULT)
                    else:
                        for tc0 in tcols:
                            nc.gpsimd.tensor_tensor(
                                out=p_ring[:, scol + tc0:scol + tc0 + 128],
                                in0=p_ring[:, scol + tc0:scol + tc0 + 128],
                                in1=tri_sb, op=MULT)
                emit_l2(h, ci % len(head_chunks), rc0)
                for si in range(n):
                    for p in slots_h[cs + si]:
                        process_piece(h, p, rc0 + si * 512 + p["so"])
                for si in range(n):
                    sidx = cs + si
                    oi = ofs_idx.get(sidx)
                    if oi is None:
                        continue
                    if ofs_idx.get(sidx + 1) is not None:
                        continue  # emitted below as a pair with its successor
                    c0 = rc0 + si * 512
                    if ofs_idx.get(sidx - 1) is not None:
                        nc.sync.dma_start(
                            pofs[h, oi - 1:oi + 1].rearrange("t p c -> p t c"),
                            p_ring[:, c0 - 512:c0 + 512].rearrange(
                                "p (t c) -> p t c", t=2))
                    else:
                        nc.sync.dma_start(pofs[h, oi], p_ring[:, c0:c0 + 512])
                if (cs, n) == head_chunks[6] and h + 1 < n_heads:
                    prep_head(h + 1)

    nc.compile()
    return nc


_NC_CACHE = None


def _get_nc():
    global _NC_CACHE
    if _NC_CACHE is None:
        _NC_CACHE = _build()
    return _NC_CACHE


def _make_in_maps(query_layer, key_layer, value_layer):
    q = np.asarray(query_layer, dtype=np.float32).reshape(B * H, S, D)
    k = np.asarray(key_layer, dtype=np.float32).reshape(B * H, S, D)
    v = np.asarray(value_layer, dtype=np.float32).reshape(B * H, S, D)
    bf = ml_dtypes.bfloat16
    qf_t = np.ascontiguousarray(q.transpose(0, 2, 1)).astype(bf)
    kf_t = np.ascontiguousarray(k.transpose(0, 2, 1)).astype(bf)
    vf = v.astype(bf)
    r = np.arange(128)
    tri_np = (r[None, :] >= r[:, None]).astype(bf)
    in_maps = []
    for c in range(N_CORES):
        sl = slice(c * HPC, (c + 1) * HPC)
        in_maps.append({"qt": qf_t[sl], "kt": kf_t[sl], "v": vf[sl],
                        "tri": tri_np})
    return in_maps


def _ofs_pieces():
    """(slot_ofs_index, j, qo, w, so) for every offset piece."""
    slots_h, _, _, ofs_slots = _plan_head()
    ofs_idx = {s: i for i, s in enumerate(ofs_slots)}
    res = []
    for si, sl in enumerate(slots_h):
        for p in sl:
            if p["offset"]:
                res.append((ofs_idx[si], p["j"], p["qo"], p["w"], p["so"]))
    return res


_OFS_PIECES = None


def kernel(query_layer, key_layer, value_layer, attention_mask):
    """Full-input causal attention; returns [b, s, h*d] float32."""
    global _OFS_PIECES
    if _OFS_PIECES is None:
        _OFS_PIECES = _ofs_pieces()
    in_maps = _make_in_maps(query_layer, key_layer, value_layer)
    nc = _get_nc()
    res = run_bass_kernel_spmd(nc, in_maps, core_ids=list(range(N_CORES)))

    ctx = np.concatenate([res.results[c]["out"] for c in range(N_CORES)],
                         axis=0).astype(np.float32)         # [64, 128, 2048]
    l2 = np.concatenate([res.results[c]["l2o"] for c in range(N_CORES)],
                        axis=0).astype(np.float32)          # [64, 4, 128, 512]
    po = np.concatenate([res.results[c]["pofs"] for c in range(N_CORES)],
                        axis=0).astype(np.float32)          # [64, 6, 128, 512]

    l = l2.sum(axis=2)                                      # [64, 4, 512]
    for (oi, j, qo, w, so) in _OFS_PIECES:
        l[:, j, qo:qo + w] += po[:, oi, :, so:so + w].sum(axis=1)
    ctx /= l.reshape(B * H, 1, S)
    return np.ascontiguousarray(
        ctx.reshape(B, H, D, S).transpose(0, 3, 1, 2)).reshape(B, S, H * D)


# revision 3
# speedup vs baseline: 1.0126x; 1.0126x over previous
"""Causal MHA (b=2,h=32,s=2048,d=128) on 8 TRN2 cores — v3.

Device computes unnormalized attention: ctx_u^T = exp(S^T/sqrt(d)) @ V and
partial softmax denominators; the host does the final (cheap, O(b s h d))
normalization ctx_u / l. This removes the per-q-block cleanup/offset
ones-matmuls from the PE, the reciprocal and final multiply from the DVE,
and frees the l PSUM bank so S chunks are uniform [128,1536] double-buffered
(96 ACT insts/core).

Device outputs per head:
  out   [128, 2048] bf16 — unnormalized ctx^T
  l2o   [4, 128, 512] bf16 — DVE fold-tree partial sums of the full-width
        P slots per q-block (host: l_j = sum over partitions + offset part)
  pofs  [6, 128, 512] bf16 — the 6 slots holding offset (384/128/256) pieces
        (host adds their column sums into l at the pieces' q-offsets)

Per-head stream: 34 slots x 512 packed S^T cols (see _plan_head).
"""
import math
import sys

if '/opt/trn_rl_repo' not in sys.path:
    sys.path.insert(0, '/opt/trn_rl_repo')

import numpy as np
import ml_dtypes

import concourse.bass as bass
import concourse.tile as tile
from concourse import mybir, bacc
from concourse.bass_utils import run_bass_kernel_spmd

F32 = mybir.dt.float32
BF16 = mybir.dt.bfloat16
EXP = mybir.ActivationFunctionType.Exp
MULT = mybir.AluOpType.mult
ADD = mybir.AluOpType.add

B, H, S, D = 2, 32, 2048, 128
N_CORES = 8
HPC = (B * H) // N_CORES
QB = 512
NQB = S // QB
NKT = S // 128
SCALE = 1.0 / math.sqrt(D)
NSLOT = 34
RING = NSLOT * 512
NOFS = 6                     # offset-piece slots per head, DMA'd for host


def _plan_head():
    slots, cur = [], []
    cur_w = 0
    tri_slots = {}
    runs = {}
    ofs_slots = []           # slot indices DMA'd to pofs

    def put(j, t, qo, w, defer=False):
        nonlocal cur, cur_w
        cur.append(dict(j=j, t=t, qo=qo, w=w, so=cur_w, defer=defer,
                        offset=(w < 512)))
        if t >= 4 * j:
            tri_slots.setdefault(len(slots), []).append(cur_w)
        if w < 512 and (not ofs_slots or ofs_slots[-1] != len(slots)):
            ofs_slots.append(len(slots))
        cur_w += w
        assert cur_w <= 512
        if cur_w == 512:
            slots.append(cur)
            cur, cur_w = [], 0

    for j in range(NQB):
        base = 4 * j
        run_start = len(slots)
        for t in range(base):
            put(j, t, 0, 512)
        put(j, base, 0, 512)
        runs[j] = (run_start, len(slots) - run_start)
        put(j, base + 1, 128, 384)
        put(j, base + 3, 384, 128)
        if j in (0, 2):
            put(j, base + 2, 256, 256)
            jn = j + 1
            put(jn, 4 * jn + 2, 256, 256, defer=True)
    assert cur_w == 0 and len(slots) == NSLOT and len(ofs_slots) == NOFS

    order = [p for sl in slots for p in sl]
    last = {}
    for i, p in enumerate(order):
        last[p["j"]] = i
    for i, p in enumerate(order):
        p["stop"] = (last[p["j"]] == i)
    return slots, runs, tri_slots, ofs_slots


def _build(n_heads=HPC, la_chunks=2):
    nc = bacc.Bacc("TRN2", target_bir_lowering=False, debug=False,
                   num_devices=N_CORES)
    qt = nc.dram_tensor("qt", [n_heads, 128, S], BF16, kind="ExternalInput")
    kt = nc.dram_tensor("kt", [n_heads, 128, S], BF16, kind="ExternalInput")
    v = nc.dram_tensor("v", [n_heads, S, D], BF16, kind="ExternalInput")
    tri = nc.dram_tensor("tri", [128, 128], BF16, kind="ExternalInput")
    out = nc.dram_tensor("out", [n_heads, 128, S], BF16, kind="ExternalOutput")
    l2o = nc.dram_tensor("l2o", [n_heads, NQB, 128, QB], BF16,
                         kind="ExternalOutput")
    pofs = nc.dram_tensor("pofs", [n_heads, NOFS, 128, QB], BF16,
                          kind="ExternalOutput")

    slots_h, runs_h, tri_h, ofs_slots = _plan_head()
    ofs_idx = {s: i for i, s in enumerate(ofs_slots)}
    # per-head chunks: 10x3 + 2x2 (last two padded into the 1536 tags)
    head_chunks = []
    s0 = 0
    for n in [3] * 10 + [2, 2]:
        head_chunks.append((s0, n))
        s0 += n
    assert s0 == NSLOT

    # incremental l2 accumulation: per head-chunk, the run slots it contains
    # (j, first_slot_in_chunk_rel, count, first_of_run, last_of_run)
    run_parts = {ci: [] for ci in range(len(head_chunks))}
    for j, (rs, rn) in runs_h.items():
        for ci, (cs, n) in enumerate(head_chunks):
            a = max(rs, cs)
            b = min(rs + rn, cs + n)
            if a < b:
                run_parts[ci].append(
                    (j, a - cs, b - a, a == rs, b == rs + rn))

    chunks = []
    for h in range(n_heads):
        for (cs, n) in head_chunks:
            chunks.append((h, cs, n))

    with tile.TileContext(nc) as tc:
        with (tc.tile_pool(name="heads", bufs=2) as hp,
              tc.tile_pool(name="consts", bufs=1) as cp,
              tc.tile_pool(name="pring", bufs=1) as prp,
              tc.tile_pool(name="tmps", bufs=2) as tp,
              tc.tile_pool(name="outp", bufs=2) as outp,
              tc.tile_pool(name="ps_s", bufs=1, space="PSUM") as ps_s,
              tc.tile_pool(name="ps_c", bufs=2, space="PSUM") as ps_c):
            tri_sb = cp.tile([128, 128], BF16)
            nc.sync.dma_start(tri_sb, tri[:, :])
            p_ring = prp.tile([128, RING], BF16, name="p_ring")
            # warm the PE clock ramp while head-0 inputs stream in
            warm_ps = ps_s.tile([128, 1536], F32, tag="s0", name="warm_ps")
            for _ in range(8):
                nc.tensor.matmul(warm_ps[:, :128], tri_sb, tri_sb,
                                 start=True, stop=True)

            head_sb = {}

            def prep_head(h):
                if h in head_sb:
                    return head_sb[h]
                qt_sb = hp.tile([128, S], BF16, tag="qt", name="qt_sb")
                kt_sb = hp.tile([128, S], BF16, tag="kt", name="kt_sb")
                v_sb = hp.tile([128, NKT, D], BF16, tag="v", name="v_sb")
                # first halves land first (chunk 0 needs kt<=896, qt<=1024)
                nc.sync.dma_start(kt_sb[:, :1024], kt[h, :, :1024])
                nc.sync.dma_start(qt_sb[:, :1024], qt[h, :, :1024])
                nc.sync.dma_start(kt_sb[:, 1024:], kt[h, :, 1024:])
                nc.sync.dma_start(qt_sb[:, 1024:], qt[h, :, 1024:])
                vr = v[h].rearrange("(t p) d -> p t d", p=128)
                for t0 in range(0, NKT, 8):
                    nc.sync.dma_start(v_sb[:, t0:t0 + 8, :], vr[:, t0:t0 + 8, :])
                head_sb[h] = (qt_sb, kt_sb, v_sb)
                return head_sb[h]

            def emit_s_chunk(ci):
                h, cs, n = chunks[ci]
                qt_sb, kt_sb, _ = prep_head(h)
                ci_h = ci % len(head_chunks)
                s_ps = ps_s.tile([128, 1536], F32, tag=f"s{ci_h % 2}",
                                 name="s_ps")
                for si in range(n):
                    for p in slots_h[cs + si]:
                        col = si * 512 + p["so"]
                        j, t = p["j"], p["t"]
                        nc.tensor.matmul(
                            s_ps[:, col:col + p["w"]],
                            kt_sb[:, t * 128:(t + 1) * 128],
                            qt_sb[:, j * QB + p["qo"]:j * QB + p["qo"] + p["w"]],
                            start=True, stop=True)
                return s_ps

            ctx_ps = {}
            deferred = {}
            npieces = {}
            for h in range(n_heads):
                for sl in slots_h:
                    for p in sl:
                        npieces[(h, p["j"])] = npieces.get((h, p["j"]), 0) + 1

            def emit_ctx(h, p, rcol, start):
                j, t = p["j"], p["t"]
                _, _, v_sb = head_sb[h]
                nc.tensor.matmul(
                    ctx_ps[(h, j)][:, p["qo"]:p["qo"] + p["w"]],
                    v_sb[:, t, :], p_ring[:, rcol:rcol + p["w"]],
                    start=start, stop=p["stop"])

            l2acc = {}    # (h, j) -> acc tile [128,512] bf16

            def head_stage(h):
                if h not in stage:
                    stage[h] = (
                        outp.tile([128, S], BF16, tag="ctx_out",
                                  name="ctx_stage"),
                        outp.tile([128, S], BF16, tag="l2s",
                                  name="l2_stage"))
                return stage[h]

            def emit_l2(h, ci_h, rc0):
                """Incremental l2 accumulation for this chunk's run slots.
                The run's final op writes the head's l2 staging tile."""
                for (j, rel, cnt, first, last) in run_parts[ci_h]:
                    key = (h, j)
                    c0 = rc0 + rel * 512
                    dest = head_stage(h)[1][:, j * QB:(j + 1) * QB]
                    srcs = [p_ring[:, c0 + k * 512:c0 + (k + 1) * 512]
                            for k in range(cnt)]
                    if first:
                        acc = l2acc[key] = tp.tile([128, QB], BF16, tag="acc",
                                                   name="l2_acc", bufs=3)
                        if cnt == 1:
                            if last:
                                nc.vector.tensor_copy(dest, srcs[0])
                            else:
                                nc.vector.tensor_copy(acc, srcs[0])
                            srcs = []
                        else:
                            nc.vector.tensor_tensor(out=acc, in0=srcs[0],
                                                    in1=srcs[1], op=ADD)
                            srcs = srcs[2:]
                    else:
                        acc = l2acc[key]
                        if cnt == 2:
                            tf = tp.tile([128, QB], BF16, tag="tf", name="tf")
                            nc.vector.tensor_tensor(out=tf, in0=srcs[0],
                                                    in1=srcs[1], op=ADD)
                            srcs = [tf]
                    for si, sap in enumerate(srcs):
                        o = dest if (last and si == len(srcs) - 1) else acc
                        nc.vector.tensor_tensor(out=o, in0=acc, in1=sap,
                                                op=ADD)
                    if last:
                        l2acc.pop(key, None)

            def process_piece(h, p, rcol):
                key = (h, p["j"])
                if p["defer"]:
                    deferred.setdefault(key, []).append((p, rcol))
                    finish_piece(h, p)
                    return
                if p["t"] == 0:
                    assert key not in ctx_ps
                    ctx_ps[key] = ps_c.tile([128, QB], F32, tag="ctx",
                                            name="ctx_ps")
                    emit_ctx(h, p, rcol, start=True)
                    for (dp, drcol) in deferred.pop(key, []):
                        emit_ctx(h, dp, drcol, start=False)
                else:
                    assert key in ctx_ps
                    emit_ctx(h, p, rcol, start=False)
                finish_piece(h, p)

            stage = {}

            def finish_piece(h, p):
                key = (h, p["j"])
                npieces[key] -= 1
                if npieces[key] == 0:
                    j = p["j"]
                    ctx_st, l2_st = head_stage(h)
                    nc.vector.tensor_copy(ctx_st[:, j * QB:(j + 1) * QB],
                                          ctx_ps[key][:, :])
                    del ctx_ps[key]
                    if j == NQB - 1:
                        nc.sync.dma_start(out[h], ctx_st)
                        nc.sync.dma_start(
                            l2o[h].rearrange("t p c -> p t c"),
                            l2_st.rearrange("p (t c) -> p t c", t=NQB))
                        del stage[h]

            pending = [emit_s_chunk(ci) for ci in range(min(la_chunks, len(chunks)))]

            for ci in range(len(chunks)):
                if ci + la_chunks < len(chunks):
                    pending.append(emit_s_chunk(ci + la_chunks))
                s_ps = pending.pop(0)
                h, cs, n = chunks[ci]
                rc0 = cs * 512
                nc.scalar.activation(p_ring[:, rc0:rc0 + n * 512],
                                     s_ps[:, :n * 512], EXP, scale=SCALE)
                for si in range(n):
                    tcols = tri_h.get(cs + si, [])
                    scol = rc0 + si * 512
                    if len(tcols) == 2:
                        stride = tcols[1] - tcols[0]
                        pap = bass.AP(tensor=p_ring.tensor,
                                      offset=p_ring.offset + scol + tcols[0],
                                      ap=[p_ring.ap[0], [stride, 2], [1, 128]])
                        tap = bass.AP(tensor=tri_sb.tensor,
                                      offset=tri_sb.offset,
                                      ap=[tri_sb.ap[0], [0, 2], [1, 128]])
                        nc.gpsimd.tensor_tensor(out=pap, in0=pap, in1=tap,
                                                op=MULT)
                    else:
                        for tc0 in tcols:
                            nc.gpsimd.tensor_tensor(
                                out=p_ring[:, scol + tc0:scol + tc0 + 128],
                                in0=p_ring[:, scol + tc0:scol + tc0 + 128],
                                in1=tri_sb, op=MULT)
                emit_l2(h, ci % len(head_chunks), rc0)
                for si in range(n):
                    for p in slots_h[cs + si]:
                        process_piece(h, p, rc0 + si * 512 + p["so"])
                for si in range(n):
                    sidx = cs + si
                    oi = ofs_idx.get(sidx)
                    if oi is None:
                        continue
                    if ofs_idx.get(sidx + 1) is not None:
                        continue  # emitted below as a pair with its successor
                    c0 = rc0 + si * 512
                    if ofs_idx.get(sidx - 1) is not None:
                        nc.sync.dma_start(
                            pofs[h, oi - 1:oi + 1].rearrange("t p c -> p t c"),
                            p_ring[:, c0 - 512:c0 + 512].rearrange(
                                "p (t c) -> p t c", t=2))
                    else:
                        nc.sync.dma_start(pofs[h, oi], p_ring[:, c0:c0 + 512])
                if (cs, n) == head_chunks[6] and h + 1 < n_heads:
                    prep_head(h + 1)

    nc.compile()
    return nc


_NC_CACHE = None


def _get_nc():
    global _NC_CACHE
    if _NC_CACHE is None:
        _NC_CACHE = _build()
    return _NC_CACHE


def _make_in_maps(query_layer, key_layer, value_layer):
    q = np.asarray(query_layer, dtype=np.float32).reshape(B * H, S, D)
    k = np.asarray(key_layer, dtype=np.float32).reshape(B * H, S, D)
    v = np.asarray(value_layer, dtype=np.float32).reshape(B * H, S, D)
    bf = ml_dtypes.bfloat16
    qf_t = np.ascontiguousarray(q.transpose(0, 2, 1)).astype(bf)
    kf_t = np.ascontiguousarray(k.transpose(0, 2, 1)).astype(bf)
    vf = v.astype(bf)
    r = np.arange(128)
    tri_np = (r[None, :] >= r[:, None]).astype(bf)
    in_maps = []
    for c in range(N_CORES):
        sl = slice(c * HPC, (c + 1) * HPC)
        in_maps.append({"qt": qf_t[sl], "kt": kf_t[sl], "v": vf[sl],
                        "tri": tri_np})
    return in_maps


def _ofs_pieces():
    """(slot_ofs_index, j, qo, w, so) for every offset piece."""
    slots_h, _, _, ofs_slots = _plan_head()
    ofs_idx = {s: i for i, s in enumerate(ofs_slots)}
    res = []
    for si, sl in enumerate(slots_h):
        for p in sl:
            if p["offset"]:
                res.append((ofs_idx[si], p["j"], p["qo"], p["w"], p["so"]))
    return res


_OFS_PIECES = None


def kernel(query_layer, key_layer, value_layer, attention_mask):
    """Full-input causal attention; returns [b, s, h*d] float32."""
    global _OFS_PIECES
    if _OFS_PIECES is None:
        _OFS_PIECES = _ofs_pieces()
    in_maps = _make_in_maps(query_layer, key_layer, value_layer)
    nc = _get_nc()
    res = run_bass_kernel_spmd(nc, in_maps, core_ids=list(range(N_CORES)))

    ctx = np.concatenate([res.results[c]["out"] for c in range(N_CORES)],
                         axis=0).astype(np.float32)         # [64, 128, 2048]
    l2 = np.concatenate([res.results[c]["l2o"] for c in range(N_CORES)],
                        axis=0).astype(np.float32)          # [64, 4, 128, 512]
    po = np.concatenate([res.results[c]["pofs"] for c in range(N_CORES)],
                        axis=0).astype(np.float32)          # [64, 6, 128, 512]

    l = l2.sum(axis=2)                                      # [64, 4, 512]
    for (oi, j, qo, w, so) in _OFS_PIECES:
        l[:, j, qo:qo + w] += po[:, oi, :, so:so + w].sum(axis=1)
    ctx /= l.reshape(B * H, 1, S)
    return np.ascontiguousarray(
        ctx.reshape(B, H, D, S).transpose(0, 3, 1, 2)).reshape(B, S, H * D)
